# revision 1
# baseline (speedup 1.0000x reference)
import numpy as np
import jax
import jax.numpy as jnp

# Problem dims (hardcoded per spec: nn_LocalPointDecoder)
B, Q, P, CD, H, NB = 2, 8192, 4096, 128, 256, 5
VAR = 0.1 ** 2
NC = 8           # NeuronCores
QS = Q // NC     # query slice per core (sharding_hint: shard Q, replicate pp/fea/params)


def _forward(p, pp, fea, fc_p_w, fc_p_b, fc_c_w, fc_c_b,
             blk0_w, blk0_b, blk1_w, blk1_b, fc_out_w, fc_out_b):
    # p: (B, QS, 3) local query slice; everything else replicated.
    diff = pp[:, None, :, :] - p[:, :, None, :]           # (B, QS, P, 3)
    dist = jnp.sqrt(jnp.sum(diff * diff, axis=3)) + 1e-5  # (B, QS, P)
    w = jnp.exp(-(dist * dist) / VAR)
    w = w / jnp.sum(w, axis=2, keepdims=True)
    c = jnp.einsum('bqp,bpc->bqc', w, fea)                # (B, QS, CD)

    net = p @ fc_p_w + fc_p_b                             # (B, QS, H)
    for i in range(NB):
        net = net + c @ fc_c_w[i] + fc_c_b[i]
        h = jax.nn.relu(net)
        dx = jax.nn.relu(h @ blk0_w[i] + blk0_b[i]) @ blk1_w[i] + blk1_b[i]
        net = net + dx
    return (jax.nn.relu(net) @ fc_out_w + fc_out_b)[..., 0]  # (B, QS)


_pmapped = jax.pmap(_forward, in_axes=(0,) + (None,) * 12)


def _numpy_fallback(p, pp, fea, fc_p_w, fc_p_b, fc_c_w, fc_c_b,
                    blk0_w, blk0_b, blk1_w, blk1_b, fc_out_w, fc_out_b):
    out = np.empty((B, Q), np.float32)
    relu = lambda x: np.maximum(x, 0.0)
    for b in range(B):
        for q0 in range(0, Q, 1024):
            ps = p[b, q0:q0 + 1024]                               # (q,3)
            diff = pp[b][None, :, :] - ps[:, None, :]             # (q,P,3)
            dist = np.sqrt(np.sum(diff * diff, axis=2)) + 1e-5
            w = np.exp(-(dist * dist) / VAR)
            w = w / np.sum(w, axis=1, keepdims=True)
            c = w @ fea[b]                                        # (q,CD)
            net = ps @ fc_p_w + fc_p_b
            for i in range(NB):
                net = net + c @ fc_c_w[i] + fc_c_b[i]
                h = relu(net)
                dx = relu(h @ blk0_w[i] + blk0_b[i]) @ blk1_w[i] + blk1_b[i]
                net = net + dx
            out[b, q0:q0 + 1024] = (relu(net) @ fc_out_w + fc_out_b)[:, 0]
    return out


def kernel(p, pp, fea, fc_p_w, fc_p_b, fc_c_w, fc_c_b,
           blk0_w, blk0_b, blk1_w, blk1_b, fc_out_w, fc_out_b):
    args = (pp, fea, fc_p_w, fc_p_b, fc_c_w, fc_c_b,
            blk0_w, blk0_b, blk1_w, blk1_b, fc_out_w, fc_out_b)
    try:
        # shard Q across the 8 cores: (B, Q, 3) -> (NC, B, QS, 3)
        p_sh = np.ascontiguousarray(
            np.asarray(p).reshape(B, NC, QS, 3).transpose(1, 0, 2, 3))
        out = np.asarray(_pmapped(p_sh, *args))                   # (NC, B, QS)
        return np.ascontiguousarray(
            out.transpose(1, 0, 2).reshape(B, Q)).astype(np.float32)
    except Exception:
        return _numpy_fallback(p, pp, fea, *args)
